# revision 1
# baseline (speedup 1.0000x reference)
"""Trainium2 Bass kernel: 3x3 valid cross-correlation on a [4096, 8192] fp32 image.

Strategy: row-shard X across 8 NeuronCores (512 output rows each, with a
2-row halo sliced host-side).  Per core, the 3x3 conv is computed as three
PSUM-accumulated fp32r band matmuls: for each column shift dj, a banded
stationary matrix B_dj[q, p] = w[q-p, dj] contracts the partition (row)
dimension, while the moving operand is the input tile column-shifted by dj.
fp32r streams at bf16 speed on the PE (1 cycle/row for moving free dim
>= 256) and is exact for inputs pre-rounded to fp32r's 12-bit mantissa
(done host-side, ~1e-4 relative rounding on the inputs).

Tiling per core: 5 row tiles of [126,126,126,126,8] output rows (input
rows+2 partitions), 16 PSUM column chunks of 512 (last 510).  PSUM->SBUF
copy (+bias) alternates between ScalarE and VectorE.  DMA: one 4 MB load
and one 4 MB store per row tile -> ~33.5 MB of HBM traffic per core,
which is the memory-bound roofline for this problem.
"""

import sys

for _p in ("/opt/trn_rl_repo", "/root/.axon_site/_ro/trn_rl_repo"):
    if _p not in sys.path:
        sys.path.append(_p)

from contextlib import ExitStack

import numpy as np

import concourse.bass as bass  # noqa: F401  (registers engine classes)
import concourse.tile as tile
from concourse import bacc, mybir
from concourse.bass_utils import run_bass_kernel_spmd

N_CORES = 8
H, W = 4096, 8192
KH, KW = 3, 3
OH, OW = H - KH + 1, W - KW + 1  # 4094 x 8190

ROWS_PER_CORE = 512           # output rows computed per core (core 7: 510 valid)
IN_ROWS = ROWS_PER_CORE + KH - 1  # 514 input rows per core
MTILE = 126                   # output rows per row tile (input rows = MTILE+2)
FDIM = 512                    # PSUM chunk width (one bank of fp32)

_F32 = mybir.dt.float32
_F32R = mybir.dt.float32r


def _round_fp32r(x: np.ndarray) -> np.ndarray:
    """Round fp32 to fp32r's representable set (round-to-nearest-even at
    mantissa bit 12), matching neuron_dtypes.static_cast_fp32_to_fp32r."""
    b = np.ascontiguousarray(x).view(np.uint32).astype(np.uint64)
    lsb = (b >> 12) & 1
    r = (b + 0x7FF + lsb) & 0xFFFFF000
    return r.astype(np.uint32).view(np.float32).reshape(x.shape)


def _row_tiles():
    tiles = []
    r0 = 0
    while r0 < ROWS_PER_CORE:
        rows = min(MTILE, ROWS_PER_CORE - r0)
        tiles.append((r0, rows))
        r0 += rows
    return tiles


def _col_chunks():
    chunks = []
    c0 = 0
    while c0 < OW:
        cols = min(FDIM, OW - c0)
        chunks.append((c0, cols))
        c0 += cols
    return chunks


def _build_program():
    nc = bacc.Bacc("TRN2", target_bir_lowering=False, debug=False,
                   num_devices=N_CORES)
    x_in = nc.dram_tensor("x", [IN_ROWS, W], _F32R, kind="ExternalInput").ap()
    bands_in = nc.dram_tensor("bands", [MTILE + 2, KW * MTILE], _F32R,
                              kind="ExternalInput").ap()
    bias_in = nc.dram_tensor("bias", [128, 1], _F32, kind="ExternalInput").ap()
    y_out = nc.dram_tensor("y", [ROWS_PER_CORE, OW], _F32,
                           kind="ExternalOutput").ap()

    with tile.TileContext(nc) as tc, ExitStack() as ctx:
        const_pool = ctx.enter_context(tc.tile_pool(name="const", bufs=1))
        xpool = ctx.enter_context(tc.tile_pool(name="xt", bufs=2))
        opool = ctx.enter_context(tc.tile_pool(name="ot", bufs=2))
        pspool = ctx.enter_context(tc.tile_pool(name="ps", bufs=6, space="PSUM"))

        bands = const_pool.tile([MTILE + 2, KW * MTILE], _F32R)
        nc.sync.dma_start(bands[:], bands_in[:, :])
        bias_t = const_pool.tile([128, 1], _F32)
        nc.sync.dma_start(bias_t[:], bias_in[:, :])

        for ti, (r0, rows) in enumerate(_row_tiles()):
            kin = rows + KH - 1
            xt = xpool.tile([MTILE + 2, W], _F32R)
            nc.sync.dma_start(xt[0:kin, :], x_in[r0:r0 + kin, :])

            ot = opool.tile([MTILE, OW], _F32)
            for ci, (c0, cols) in enumerate(_col_chunks()):
                ps = pspool.tile([MTILE, FDIM], _F32)
                for dj in range(KW):
                    nc.tensor.matmul(
                        ps[0:rows, 0:cols],
                        bands[0:kin, dj * MTILE:dj * MTILE + rows],
                        xt[0:kin, c0 + dj:c0 + dj + cols],
                        start=(dj == 0),
                        stop=(dj == KW - 1),
                    )
                # PSUM -> SBUF with bias add; alternate engines.
                if ci % 2 == 0:
                    nc.scalar.add(ot[0:rows, c0:c0 + cols], ps[0:rows, 0:cols],
                                  bias_t[0:rows, 0:1])
                else:
                    nc.vector.tensor_scalar_add(ot[0:rows, c0:c0 + cols],
                                                ps[0:rows, 0:cols],
                                                bias_t[0:rows, 0:1])

            nc.sync.dma_start(y_out[r0:r0 + rows, :], ot[0:rows, :])

    nc.compile()
    return nc


_NC_CACHE = None


def _get_program():
    global _NC_CACHE
    if _NC_CACHE is None:
        _NC_CACHE = _build_program()
    return _NC_CACHE


def _make_bands(weight: np.ndarray) -> np.ndarray:
    """bands[q, dj*MTILE + p] = weight[q - p, dj] for q - p in [0, KH)."""
    k = MTILE + 2
    bands = np.zeros((k, KW * MTILE), dtype=np.float32)
    for dj in range(KW):
        for di in range(KH):
            idx = np.arange(MTILE)
            bands[idx + di, dj * MTILE + idx] = weight[di, dj]
    return bands


def _shard_inputs(X: np.ndarray, weight: np.ndarray, bias: np.ndarray):
    Xr = _round_fp32r(X.astype(np.float32, copy=False))
    bands = _round_fp32r(_make_bands(weight.astype(np.float32, copy=False)))
    bias_col = np.ascontiguousarray(
        np.broadcast_to(bias.astype(np.float32, copy=False).reshape(1, 1),
                        (128, 1)))
    in_maps = []
    for c in range(N_CORES):
        r0 = c * ROWS_PER_CORE
        r1 = min(r0 + IN_ROWS, H)
        xs = Xr[r0:r1]
        if xs.shape[0] < IN_ROWS:  # pad last core's halo with zeros
            pad = np.zeros((IN_ROWS - xs.shape[0], W), dtype=np.float32)
            xs = np.concatenate([xs, pad], axis=0)
        in_maps.append({
            "x": np.ascontiguousarray(xs),
            "bands": bands,
            "bias": bias_col,
        })
    return in_maps


def _gather(results) -> np.ndarray:
    out = np.empty((OH, OW), dtype=np.float32)
    for c in range(N_CORES):
        r0 = c * ROWS_PER_CORE
        rows = min(ROWS_PER_CORE, OH - r0)
        out[r0:r0 + rows] = results[c]["y"][0:rows]
    return out


def run(X, weight, bias, trace=False, **spmd_kwargs):
    """Full-input entry point; returns (output, BassKernelResults)."""
    nc = _get_program()
    in_maps = _shard_inputs(X, weight, bias)
    res = run_bass_kernel_spmd(nc, in_maps, list(range(N_CORES)),
                               trace=trace, **spmd_kwargs)
    return _gather(res.results), res


def kernel(X, weight, bias):
    out, _ = run(X, weight, bias)
    return out
